# revision 7
# baseline (speedup 1.0000x reference)
"""Trainium2 Bass kernel for nn_AttnDecoderRNN (Bahdanau attention + GRU decoder).

B=256, S=64, H=1024, V=32000, T=10 decode steps, 8 NeuronCores.
Sharding: pure data-parallel over batch (32 rows/core, zero collectives).

Per-core program:
  phase 0: UkT = (keys @ Ua^T + Ua_b)^T   (PE, bf16)  -> spilled to DRAM
           gi_emb = emb_all @ W_ih[:, :H]^T + b_fold  (PE)  -> spilled to DRAM
  phase 1: 10 sequential steps of additive attention + GRU (T-layout:
           h on partitions for attention; gates in [batch, j] layout)
  phase 2: logits = Hs @ out_w^T + out_b (bf16, out_b folded as a 9th
           K-chunk), online sum-exp for log_softmax (no max needed:
           logit range is +-3), then dec = logits - lse via ACT bias.

Host side does only layout work: shard, transpose, dtype-cast, embedding
GATHER (index lookup, no FLOPs), and final concat/permutes.
"""

import numpy as np
import ml_dtypes
from contextlib import ExitStack

import concourse.bass as bass
import concourse.bacc as bacc
import concourse.tile as tile
from concourse import mybir
from concourse.masks import make_identity
from concourse.alu_op_type import AluOpType
from concourse.bass_utils import run_bass_kernel_spmd

F32 = mybir.dt.float32
BF16 = mybir.dt.bfloat16
AF = mybir.ActivationFunctionType
AX = mybir.AxisListType
OP = AluOpType
BF = ml_dtypes.bfloat16

B, S, H, V, T = 256, 64, 1024, 32000, 10
NC = 8
BL = B // NC            # 32 batch rows per core
R = T * BL              # 320 decoded rows per core
KC = H // 128           # 8 k-chunks of the hidden dim
NBS = BL * S            # 2048 (b, s) pairs per core
VT = 512                # vocab tile
NV = 63                 # vocab tiles (padded V)
VP = NV * VT            # 32256 padded vocab
G3 = 3 * H              # 3072 gate width

_CACHE = {}


def _build():
    nc = bacc.Bacc(None, target_bir_lowering=False)
    P = {}

    def par(name, shape, dt, out=False):
        P[name] = nc.declare_dram_parameter(name, list(shape), dt, isOutput=out)
        return P[name]

    # ---- inputs (per core) ----
    par("keysT", [H, NBS], BF16)          # encoder outputs, (h, b*64+s)
    par("uaT", [H, H], BF16)              # Ua_w^T
    par("waT", [H, H], BF16)              # Wa_w^T
    par("wiheT", [H, G3], BF16)           # gru_w_ih[:, :H]^T
    par("wihcT", [H, G3], BF16)           # gru_w_ih[:, H:]^T
    par("whhT", [H, G3], BF16)            # gru_w_hh^T
    par("vaTrep", [H, 128], BF16)         # Va_w[0] replicated over 128 cols
    par("embT", [H, R], BF16)             # gathered embeddings, col = t*32+b
    par("h0T", [H, BL], BF16)
    par("h0f", [BL, H], F32)
    par("ua_bT", [H, 1], F32)
    par("wa_bT", [H, 1], F32)
    par("bfold", [128, G3], F32)          # (b_ih+b_hh)[:2H] ++ b_ih[2H:], row-rep
    par("bhn", [BL, H], F32)              # b_hh[2H:] replicated rows
    par("owT", [NV, 128, KC + 1, VT], BF16)  # out_w^T tiled; chunk 8 row0=out_b
    # ---- outputs ----
    par("dec", [R, V], F32, out=True)
    par("hlast", [BL, H], F32, out=True)
    par("attnb", [T, NBS], F32, out=True)
    # ---- internal DRAM ----
    ukT_d = nc.dram_tensor("ukT_d", [H, NBS], BF16)
    ge_d = nc.dram_tensor("ge_d", [R, G3], BF16)

    with tile.TileContext(nc) as tc, ExitStack() as ctx:
        # ----- cross-phase resident tiles -----
        consts = ctx.enter_context(tc.tile_pool(name="consts", bufs=1))
        wihc_s = consts.tile([128, KC, G3], BF16)
        nc.sync.dma_start(out=wihc_s, in_=P["wihcT"][:, :].rearrange("(c p) n -> p c n", p=128))
        whh_s = consts.tile([128, KC, G3], BF16)
        nc.sync.dma_start(out=whh_s, in_=P["whhT"][:, :].rearrange("(c p) n -> p c n", p=128))
        va_s = consts.tile([128, KC, 128], BF16)
        nc.sync.dma_start(out=va_s, in_=P["vaTrep"][:, :].rearrange("(c p) n -> p c n", p=128))
        h0T_s = consts.tile([128, KC, BL], BF16)
        nc.sync.dma_start(out=h0T_s, in_=P["h0T"][:, :].rearrange("(c p) n -> p c n", p=128))
        bhn_s = consts.tile([BL, H], F32)
        nc.sync.dma_start(out=bhn_s, in_=P["bhn"][:, :])
        uab_s = consts.tile([128, KC], F32)
        nc.sync.dma_start(out=uab_s, in_=P["ua_bT"][:, :].rearrange("(c p) o -> p (c o)", p=128))
        wab_s = consts.tile([128, KC], F32)
        nc.sync.dma_start(out=wab_s, in_=P["wa_bT"][:, :].rearrange("(c p) o -> p (c o)", p=128))
        ident = consts.tile([128, 128], F32)
        make_identity(nc, ident)
        # h-state history: slot t holds h_{t+1}^T (bf16); chunk KC is the
        # constant ones-row used to fold out_b into the phase-2 matmul.
        hsT = consts.tile([128, KC + 1, R], BF16)
        nc.vector.memset(hsT[:, KC, :], 0.0)
        nc.vector.memset(hsT[0:1, KC, :], 1.0)

        # =================== phase 0: UkT and gi_emb ===================
        with tc.tile_pool(name="p0", bufs=1) as p0, \
             tc.tile_pool(name="p0ps", bufs=1, space="PSUM") as p0ps:
            ua_s = p0.tile([128, KC, H], BF16)
            nc.sync.dma_start(out=ua_s, in_=P["uaT"][:, :].rearrange("(c p) n -> p c n", p=128))
            keys_s = p0.tile([128, KC, NBS], BF16)
            nc.sync.dma_start(out=keys_s, in_=P["keysT"][:, :].rearrange("(c p) n -> p c n", p=128))
            for m in range(KC):          # output h-chunk of UkT
                uk_ps = p0ps.tile([128, 4, VT], F32, tag="ukps")
                for n in range(4):       # 512-wide slices of the 2048 bs cols
                    for k in range(KC):
                        nc.tensor.matmul(uk_ps[:, n], ua_s[:, k, m * 128:(m + 1) * 128],
                                         keys_s[:, k, n * VT:(n + 1) * VT],
                                         start=(k == 0), stop=(k == KC - 1))
                uk_sb = p0.tile([128, NBS], BF16, tag="uksb", bufs=2)
                nc.scalar.activation(uk_sb, uk_ps.rearrange("p a b -> p (a b)"),
                                     AF.Identity, bias=uab_s[:, m:m + 1])
                nc.sync.dma_start(out=ukT_d[m * 128:(m + 1) * 128, :], in_=uk_sb)

            emb_s = p0.tile([128, KC, R], BF16)
            nc.sync.dma_start(out=emb_s, in_=P["embT"][:, :].rearrange("(c p) n -> p c n", p=128))
            bfold_s = p0.tile([128, G3], F32)
            nc.sync.dma_start(out=bfold_s, in_=P["bfold"][:, :])
            for mi, (r0, rr) in enumerate([(0, 128), (128, 128), (256, 64)]):
                for n in range(3):       # 1024-wide slices of 3072 gates
                    ge_ps = p0ps.tile([128, 2, VT], F32, tag="geps", bufs=2,
                                      name=f"geps_{mi}_{n}")
                    # stream wiheT per k-chunk
                    for k in range(KC):
                        wt = p0.tile([128, G3], BF16, tag="wihe", bufs=2, name=f"wihe_{mi}_{n}_{k}")
                        nc.sync.dma_start(out=wt, in_=P["wiheT"][k * 128:(k + 1) * 128, :])
                        nc.tensor.matmul(ge_ps[:rr, 0], emb_s[:, k, r0:r0 + rr],
                                         wt[:, n * 1024:n * 1024 + VT],
                                         start=(k == 0), stop=(k == KC - 1))
                        nc.tensor.matmul(ge_ps[:rr, 1], emb_s[:, k, r0:r0 + rr],
                                         wt[:, n * 1024 + VT:(n + 1) * 1024],
                                         start=(k == 0), stop=(k == KC - 1))
                    ge_sb = p0.tile([128, 1024], BF16, tag="gesb", bufs=2, name=f"ge_{mi}_{n}")
                    nc.vector.scalar_tensor_tensor(
                        out=ge_sb[:rr], in0=ge_ps[:rr].rearrange("p a b -> p (a b)"),
                        scalar=0.0, in1=bfold_s[:rr, n * 1024:(n + 1) * 1024],
                        op0=OP.add, op1=OP.add)
                    nc.sync.dma_start(out=ge_d[r0:r0 + rr, n * 1024:(n + 1) * 1024], in_=ge_sb[:rr])

        # =================== phase 1: 10 decode steps ===================
        with tc.tile_pool(name="p1", bufs=1) as p1, \
             tc.tile_pool(name="p1ps", bufs=1, space="PSUM") as p1ps:
            h_f32 = None
            for t in range(T):
                hT_prev = h0T_s if t == 0 else hsT[:, 0:KC, (t - 1) * BL:t * BL]

                # --- wq = h @ Wa^T  ([BL, H]), then transpose to wqT ---
                wa_t = p1.tile([128, 2, H], BF16, tag="wat", bufs=2, name=f"wa_{t}")
                wq_ps = p1ps.tile([BL, 2, VT], F32, tag="small", bufs=2, name=f"wqps_{t}")
                for kh in range(4):   # stream waT 2 k-chunks at a time
                    wa_t = p1.tile([128, 2, H], BF16, tag="wat", bufs=2, name=f"wa_{t}_{kh}")
                    nc.sync.dma_start(out=wa_t, in_=P["waT"][kh * 256:(kh + 1) * 256, :]
                                      .rearrange("(c p) n -> p c n", p=128))
                    for kk in range(2):
                        k = kh * 2 + kk
                        hk = hT_prev[:, k] if t == 0 else hT_prev[:, k, :]
                        for n in range(2):
                            nc.tensor.matmul(wq_ps[:, n], hk, wa_t[:, kk, n * VT:(n + 1) * VT],
                                             start=(k == 0), stop=(k == KC - 1))
                wq_sb = p1.tile([BL, H], F32, tag="wqsb", bufs=1, name=f"wqsb_{t}")
                nc.scalar.activation(wq_sb, wq_ps.rearrange("p a b -> p (a b)"), AF.Copy)
                # transpose 8x [32,128] -> [128,32] through one 4-bank psum
                tr_ps = p1ps.tile([128, 4, VT], F32, tag="big", bufs=1, name=f"wqtr_{t}")
                for c in range(KC):
                    nc.tensor.transpose(
                        tr_ps[:, c // 2, (c % 2) * BL:((c % 2) + 1) * BL],
                        wq_sb[:, c * 128:(c + 1) * 128], ident[:BL, :BL])
                wqT = p1.tile([128, KC, BL], F32, tag="wqT", bufs=2, name=f"wqT_{t}")
                for c in range(KC):
                    nc.scalar.activation(wqT[:, c, :], tr_ps[:, c // 2, (c % 2) * BL:((c % 2) + 1) * BL],
                                         AF.Identity, bias=wab_s[:, c:c + 1])

                # --- attention: e = tanh(Uk + wq), scores = Va.e (replicated) ---
                sc_ps = p1ps.tile([128, 4, VT], F32, tag="big", bufs=1, name=f"scps_{t}")
                e_chunks = []
                for c in range(KC):
                    uk_t = p1.tile([128, NBS], BF16, tag="ukst", bufs=2, name=f"uk_{t}_{c}")
                    nc.sync.dma_start(out=uk_t, in_=ukT_d[c * 128:(c + 1) * 128, :])
                    pre = p1.tile([128, BL, S], BF16, tag="pre", bufs=2, name=f"pre_{t}_{c}")
                    nc.vector.scalar_tensor_tensor(
                        out=pre, in0=uk_t.rearrange("p (b s) -> p b s", b=BL),
                        scalar=0.0, in1=wqT[:, c, :].unsqueeze(-1).broadcast_to([128, BL, S]),
                        op0=OP.add, op1=OP.add)
                    e_sb = p1.tile([128, NBS], BF16, tag="esb", bufs=2, name=f"e_{t}_{c}")
                    nc.scalar.activation(e_sb, pre.rearrange("p b s -> p (b s)"), AF.Tanh)
                    for n in range(4):
                        nc.tensor.matmul(sc_ps[:, n], va_s[:, c, :],
                                         e_sb[:, n * VT:(n + 1) * VT],
                                         start=(c == 0), stop=(c == KC - 1))
                # exp (no max subtraction needed: |scores| <= ~6)
                exp_sb = p1.tile([128, NBS], BF16, tag="expsb", bufs=1, name=f"exp_{t}")
                nc.scalar.activation(exp_sb, sc_ps.rearrange("p a b -> p (a b)"), AF.Exp)
                sums = p1.tile([128, BL], F32, tag="sums", bufs=2, name=f"sums_{t}")
                nc.vector.reduce_sum(sums, exp_sb.rearrange("p (b s) -> p b s", b=BL), axis=AX.X)
                recip = p1.tile([128, BL], F32, tag="recip", bufs=2, name=f"recip_{t}")
                nc.vector.reciprocal(recip, sums)

                # attentions output row (partition 0 only)
                attn_sb = p1.tile([1, NBS], F32, tag="rzsb", bufs=1, name=f"attn_{t}")
                nc.vector.scalar_tensor_tensor(
                    out=attn_sb.rearrange("p (b s) -> p b s", b=BL),
                    in0=exp_sb[0:1, :].rearrange("p (b s) -> p b s", b=BL),
                    scalar=0.0, in1=recip[0:1, :].unsqueeze(-1).broadcast_to([1, BL, S]),
                    op0=OP.add, op1=OP.mult)
                nc.sync.dma_start(out=P["attnb"][t:t + 1, :], in_=attn_sb)

                # --- ctx^T[h, b] = sum_s keys*exp * recip ---
                keys_t_tiles = []
                ctx_red = p1.tile([128, KC, BL], F32, tag="ctxred", bufs=2, name=f"ctxred_{t}")
                for c in range(KC):
                    kt = p1.tile([128, NBS], BF16, tag="keyst", bufs=2, name=f"keys_{t}_{c}")
                    nc.sync.dma_start(out=kt, in_=P["keysT"][c * 128:(c + 1) * 128, :])
                    mt = p1.tile([128, NBS], BF16, tag="mult", bufs=1, name=f"mt_{t}_{c}")
                    nc.vector.tensor_tensor(out=mt, in0=kt, in1=exp_sb, op=OP.mult)
                    nc.vector.reduce_sum(ctx_red[:, c, :], mt.rearrange("p (b s) -> p b s", b=BL), axis=AX.X)
                ctxT = p1.tile([128, KC, BL], BF16, tag="ctxT", bufs=2, name=f"ctxT_{t}")
                nc.vector.tensor_tensor(out=ctxT, in0=ctx_red,
                                        in1=recip.unsqueeze(1).broadcast_to([128, KC, BL]),
                                        op=OP.mult)

                # --- GRU gates ---
                # rz: j 0:2048 accumulates ctx-part + h-part (2 halves x 2 banks)
                ge_t = p1.tile([BL, G3], BF16, tag="get", bufs=1, name=f"ge_{t}")
                nc.sync.dma_start(out=ge_t, in_=ge_d[t * BL:(t + 1) * BL, :])
                rz_sb = p1.tile([BL, 2 * H], F32, tag="rzsb", bufs=1, name=f"rz_{t}")
                for half in range(2):
                    rz_ps = p1ps.tile([BL, 2, VT], F32, tag="small", bufs=2, name=f"rzps_{t}_{half}")
                    for n in range(2):
                        j0 = half * 1024 + n * VT
                        for k in range(KC):
                            nc.tensor.matmul(rz_ps[:, n], ctxT[:, k, :], wihc_s[:, k, j0:j0 + VT],
                                             start=(k == 0), stop=False)
                        for k in range(KC):
                            hk = hT_prev[:, k] if t == 0 else hT_prev[:, k, :]
                            nc.tensor.matmul(rz_ps[:, n], hk, whh_s[:, k, j0:j0 + VT],
                                             start=False, stop=(k == KC - 1))
                    pre_rz = p1.tile([BL, H], F32, tag="gtmp", bufs=3, name=f"prerz_{t}_{half}")
                    nc.vector.scalar_tensor_tensor(
                        out=pre_rz, in0=rz_ps.rearrange("p a b -> p (a b)"), scalar=0.0,
                        in1=ge_t[:, half * H:(half + 1) * H], op0=OP.add, op1=OP.add)
                    nc.scalar.activation(rz_sb[:, half * H:(half + 1) * H], pre_rz, AF.Sigmoid)
                # inn / hn: j 2048:3072
                in_ps = p1ps.tile([BL, 2, VT], F32, tag="small", bufs=2, name=f"inps_{t}")
                hn_ps = p1ps.tile([BL, 2, VT], F32, tag="small", bufs=2, name=f"hnps_{t}")
                for n in range(2):
                    j0 = 2 * H + n * VT
                    for k in range(KC):
                        nc.tensor.matmul(in_ps[:, n], ctxT[:, k, :], wihc_s[:, k, j0:j0 + VT],
                                         start=(k == 0), stop=(k == KC - 1))
                    for k in range(KC):
                        hk = hT_prev[:, k] if t == 0 else hT_prev[:, k, :]
                        nc.tensor.matmul(hn_ps[:, n], hk, whh_s[:, k, j0:j0 + VT],
                                         start=(k == 0), stop=(k == KC - 1))
                pre_in = p1.tile([BL, H], F32, tag="gtmp", bufs=3, name=f"prein_{t}")
                nc.vector.scalar_tensor_tensor(
                    out=pre_in, in0=in_ps.rearrange("p a b -> p (a b)"), scalar=0.0,
                    in1=ge_t[:, 2 * H:], op0=OP.add, op1=OP.add)
                pre_hn = p1.tile([BL, H], F32, tag="gtmp", bufs=3, name=f"prehn_{t}")
                nc.vector.scalar_tensor_tensor(
                    out=pre_hn, in0=hn_ps.rearrange("p a b -> p (a b)"), scalar=0.0,
                    in1=bhn_s, op0=OP.add, op1=OP.add)
                # n = tanh(pre_in + r * pre_hn)
                rhn = p1.tile([BL, H], F32, tag="gtmp", bufs=3, name=f"rhn_{t}")
                nc.vector.tensor_tensor(out=rhn, in0=rz_sb[:, :H], in1=pre_hn, op=OP.mult)
                npre = p1.tile([BL, H], F32, tag="gtmp", bufs=3, name=f"npre_{t}")
                nc.vector.tensor_tensor(out=npre, in0=rhn, in1=pre_in, op=OP.add)
                n_sb = p1.tile([BL, H], F32, tag="nsb", bufs=1, name=f"nsb_{t}")
                nc.scalar.activation(n_sb, npre, AF.Tanh)
                # h2 = n + z*(h - n)
                hprev_f = h_f32
                if t == 0:
                    hprev_f = p1.tile([BL, H], F32, tag="hf", bufs=2, name="hf_init")
                    nc.sync.dma_start(out=hprev_f, in_=P["h0f"][:, :])
                hmn = p1.tile([BL, H], F32, tag="gtmp", bufs=3, name=f"hmn_{t}")
                nc.vector.tensor_tensor(out=hmn, in0=hprev_f, in1=n_sb, op=OP.subtract)
                zh = p1.tile([BL, H], F32, tag="gtmp", bufs=3, name=f"zh_{t}")
                nc.vector.tensor_tensor(out=zh, in0=rz_sb[:, H:], in1=hmn, op=OP.mult)
                h_new = p1.tile([BL, H], F32, tag="hf", bufs=2, name=f"hf_{t}")
                nc.vector.tensor_tensor(out=h_new, in0=n_sb, in1=zh, op=OP.add)
                h_f32 = h_new
                # transpose h_new into hsT slot t (bf16)
                htr_ps = p1ps.tile([128, 4, VT], F32, tag="big", bufs=1, name=f"htr_{t}")
                for c in range(KC):
                    nc.tensor.transpose(
                        htr_ps[:, c // 2, (c % 2) * BL:((c % 2) + 1) * BL],
                        h_new[:, c * 128:(c + 1) * 128], ident[:BL, :BL])
                for c in range(KC):
                    nc.scalar.activation(hsT[:, c, t * BL:(t + 1) * BL],
                                         htr_ps[:, c // 2, (c % 2) * BL:((c % 2) + 1) * BL],
                                         AF.Copy)
                if t == T - 1:
                    nc.sync.dma_start(out=P["hlast"][:, :], in_=h_new)

        # =================== phase 2: logits + log_softmax ===================
        MT = [(0, 128), (128, 128), (256, 64)]
        lg_d = nc.dram_tensor("lg_d", [R, VP], BF16)
        with tc.tile_pool(name="p2", bufs=1) as p2, \
             tc.tile_pool(name="p2ps", bufs=1, space="PSUM") as p2ps:
            s_acc = [p2.tile([mr, NV], F32, name=f"sacc{mi}")
                     for mi, (m0, mr) in enumerate(MT)]
            for n in range(NV):
                ow_t = p2.tile([128, KC + 1, VT], BF16, tag="owt", bufs=3, name=f"ow_{n}")
                nc.sync.dma_start(out=ow_t, in_=P["owT"][n])
                for mi, (m0, mr) in enumerate(MT):
                    lg_ps = p2ps.tile([128, VT], F32, tag="lgps", bufs=6, name=f"lgps_{n}_{mi}")
                    for k in range(KC + 1):
                        nc.tensor.matmul(lg_ps[:mr], hsT[:, k, m0:m0 + mr], ow_t[:, k, :],
                                         start=(k == 0), stop=(k == KC))
                    escr = p2.tile([128, VT], BF16, tag="escr", bufs=3, name=f"escr_{n}_{mi}")
                    nc.scalar.activation(escr[:mr], lg_ps[:mr], AF.Exp,
                                         accum_out=s_acc[mi][:, n:n + 1])
                    lg_sb = p2.tile([128, VT], BF16, tag="lgsb", bufs=4, name=f"lg_{n}_{mi}")
                    nc.vector.tensor_copy(out=lg_sb[:mr], in_=lg_ps[:mr])
                    nc.sync.dma_start(out=lg_d[m0:m0 + mr, n * VT:(n + 1) * VT],
                                      in_=lg_sb[:mr])
            # lse per m-tile, then dec = logits - lse
            for mi, (m0, mr) in enumerate(MT):
                ssum = p2.tile([mr, 1], F32, tag="ssum", bufs=2, name=f"ssum_{mi}")
                nc.vector.reduce_sum(ssum, s_acc[mi], axis=AX.X)
                nlse = p2.tile([mr, 1], F32, tag="nlse", bufs=2, name=f"nlse_{mi}")
                nc.scalar.activation(nlse, ssum, AF.Ln)
                nc.vector.tensor_scalar(out=nlse, in0=nlse, scalar1=-1.0, scalar2=None,
                                        op0=OP.mult)
                for n in range(NV):
                    ncols = VT if n < NV - 1 else (V - (NV - 1) * VT)
                    lg_in = p2.tile([128, VT], BF16, tag="lgin", bufs=4, name=f"lgi_{mi}_{n}")
                    nc.sync.dma_start(out=lg_in[:mr], in_=lg_d[m0:m0 + mr, n * VT:(n + 1) * VT])
                    dec_t = p2.tile([128, VT], F32, tag="dect", bufs=4, name=f"dec_{mi}_{n}")
                    nc.scalar.activation(dec_t[:mr, :ncols], lg_in[:mr, :ncols],
                                         AF.Identity, bias=nlse)
                    nc.sync.dma_start(out=P["dec"][m0:m0 + mr, n * VT:n * VT + ncols],
                                      in_=dec_t[:mr, :ncols])

    nc.finalize()
    return nc


def _prep(inputs):
    f32 = np.float32
    enc = np.asarray(inputs["encoder_outputs"], f32)          # [B, S, H]
    h0 = np.asarray(inputs["encoder_hidden"], f32)[0]         # [B, H]
    tgt = np.asarray(inputs["target_tensor"])                 # [B, T]
    emb = np.asarray(inputs["embedding"], f32)                # [V, H]
    Wa = np.asarray(inputs["Wa_w"], f32); Wab = np.asarray(inputs["Wa_b"], f32)
    Ua = np.asarray(inputs["Ua_w"], f32); Uab = np.asarray(inputs["Ua_b"], f32)
    Va = np.asarray(inputs["Va_w"], f32)
    wih = np.asarray(inputs["gru_w_ih"], f32); whh = np.asarray(inputs["gru_w_hh"], f32)
    bih = np.asarray(inputs["gru_b_ih"], f32); bhh = np.asarray(inputs["gru_b_hh"], f32)
    ow = np.asarray(inputs["out_w"], f32); ob = np.asarray(inputs["out_b"], f32)

    toks = np.concatenate([np.zeros((B, 1), tgt.dtype), tgt[:, :T - 1]], axis=1)  # [B, T]
    E = emb[toks.T]                                            # [T, B, H] gather

    uaT = np.ascontiguousarray(Ua.T).astype(BF)
    waT = np.ascontiguousarray(Wa.T).astype(BF)
    wihT = np.ascontiguousarray(wih.T)                         # [2H, 3H]
    wiheT = wihT[:H].astype(BF)
    wihcT = wihT[H:].astype(BF)
    whhT = np.ascontiguousarray(whh.T).astype(BF)
    vaTrep = np.repeat(Va[0][:, None], 128, axis=1).astype(BF)
    bfold = np.concatenate([(bih + bhh)[:2 * H], bih[2 * H:]])
    bfold_rep = np.tile(bfold[None, :], (128, 1)).astype(f32)
    bhn_rep = np.tile(bhh[2 * H:][None, :], (BL, 1)).astype(f32)

    OWP = np.zeros((KC + 1, 128, VP), BF)
    owT = np.ascontiguousarray(ow.T)                           # [H, V]
    OWP[:KC, :, :V] = owT.reshape(KC, 128, V).astype(BF)
    obp = np.full((VP,), -10000.0, f32)
    obp[:V] = ob
    OWP[KC, 0, :] = obp.astype(BF)
    owT_t = np.ascontiguousarray(
        OWP.reshape(KC + 1, 128, NV, VT).transpose(2, 1, 0, 3))  # [NV, 128, KC+1, VT]

    in_maps = []
    for c in range(NC):
        sl = slice(c * BL, (c + 1) * BL)
        keysT = np.ascontiguousarray(enc[sl].transpose(2, 0, 1).reshape(H, NBS)).astype(BF)
        embT = np.ascontiguousarray(E[:, sl].transpose(2, 0, 1).reshape(H, R)).astype(BF)
        h0c = np.ascontiguousarray(h0[sl])
        in_maps.append({
            "keysT": keysT, "uaT": uaT, "waT": waT, "wiheT": wiheT,
            "wihcT": wihcT, "whhT": whhT, "vaTrep": vaTrep, "embT": embT,
            "h0T": np.ascontiguousarray(h0c.T).astype(BF), "h0f": h0c,
            "ua_bT": Uab[:, None].astype(f32), "wa_bT": Wab[:, None].astype(f32),
            "bfold": bfold_rep, "bhn": bhn_rep, "owT": owT_t,
        })
    return in_maps


def _post(results):
    dec = np.stack([r["dec"].reshape(T, BL, V) for r in results])      # [NC, T, BL, V]
    dec = dec.transpose(0, 2, 1, 3).reshape(B, T, V)
    hlast = np.concatenate([r["hlast"] for r in results])[None]        # [1, B, H]
    attn = np.stack([r["attnb"].reshape(T, BL, S) for r in results])   # [NC, T, BL, S]
    attn = attn.transpose(0, 2, 1, 3).reshape(B, T, S)
    return dec, hlast, attn


def run_parts(inputs, trace=False, **kw):
    if "nc" not in _CACHE:
        _CACHE["nc"] = _build()
    nc = _CACHE["nc"]
    in_maps = _prep(inputs)
    res = run_bass_kernel_spmd(nc, in_maps, core_ids=list(range(NC)), trace=trace, **kw)
    return _post(res.results), res


def kernel(**inputs):
    (dec, hlast, attn), _ = run_parts(inputs)
    return dec, hlast, attn


# revision 13
# speedup vs baseline: 1.1218x; 1.1218x over previous
"""Trainium2 Bass kernel for nn_AttnDecoderRNN (Bahdanau attention + GRU decoder).

B=256, S=64, H=1024, V=32000, T=10 decode steps, 8 NeuronCores.
Sharding: pure data-parallel over batch (32 rows/core, zero collectives).

Per-core program:
  phase 0: UkT = (keys @ Ua^T + Ua_b)^T   (PE, bf16)  -> spilled to DRAM
           gi_emb = emb_all @ W_ih[:, :H]^T + b_fold  (PE)  -> spilled to DRAM
  phase 1: 10 sequential steps of additive attention + GRU (T-layout:
           h on partitions for attention; gates in [batch, j] layout)
  phase 2: logits = Hs @ out_w^T + out_b (bf16, out_b folded as a 9th
           K-chunk), online sum-exp for log_softmax (no max needed:
           logit range is +-3), then dec = logits - lse via ACT bias.

Host side does only layout work: shard, transpose, dtype-cast, embedding
GATHER (index lookup, no FLOPs), and final concat/permutes.
"""

import numpy as np
import ml_dtypes
from contextlib import ExitStack

import concourse.bass as bass
import concourse.bacc as bacc
import concourse.tile as tile
from concourse import mybir
from concourse.masks import make_identity
from concourse.alu_op_type import AluOpType
from concourse.bass_utils import run_bass_kernel_spmd

F32 = mybir.dt.float32
BF16 = mybir.dt.bfloat16
AF = mybir.ActivationFunctionType
AX = mybir.AxisListType
OP = AluOpType
BF = ml_dtypes.bfloat16

B, S, H, V, T = 256, 64, 1024, 32000, 10
NC = 8
BL = B // NC            # 32 batch rows per core
R = T * BL              # 320 decoded rows per core
KC = H // 128           # 8 k-chunks of the hidden dim
NBS = BL * S            # 2048 (b, s) pairs per core
VT = 512                # vocab tile
NV = 64                 # vocab tiles (padded V)
VP = NV * VT            # 32256 padded vocab
G3 = 3 * H              # 3072 gate width

_CACHE = {}


def _build():
    nc = bacc.Bacc(None, target_bir_lowering=False)
    P = {}

    def par(name, shape, dt, out=False):
        P[name] = nc.declare_dram_parameter(name, list(shape), dt, isOutput=out)
        return P[name]

    # ---- inputs (per core) ----
    par("keysT", [H, NBS], BF16)          # encoder outputs, (h, b*64+s)
    par("uaT", [H, H], BF16)              # Ua_w^T
    par("waT", [H, H], BF16)              # Wa_w^T
    par("wiheT", [H, G3], BF16)           # gru_w_ih[:, :H]^T
    par("wihcT", [H, G3], BF16)           # gru_w_ih[:, H:]^T
    par("whhT", [H, G3], BF16)            # gru_w_hh^T
    par("vaTrep", [H, 128], BF16)         # Va_w[0] replicated over 128 cols
    par("embT", [H, R], BF16)             # gathered embeddings, col = t*32+b
    par("h0T", [H, BL], BF16)
    par("h0f", [BL, H], F32)
    par("ua_bT", [H, 1], F32)
    par("wa_bT", [H, 1], F32)
    par("bfold", [128, G3], F32)          # (b_ih+b_hh)[:2H] ++ b_ih[2H:], row-rep
    par("bhn", [BL, H], F32)              # b_hh[2H:] replicated rows
    par("owT", [NV // 4, 128, KC + 1, 4 * VT], BF16)  # out_w^T tiled; chunk 8 row0=out_b
    # ---- outputs ----
    par("dec", [R, V], F32, out=True)
    par("hlast", [BL, H], F32, out=True)
    par("attnb", [T, NBS], F32, out=True)
    # ---- internal DRAM ----
    ukT_d = nc.dram_tensor("ukT_d", [H, NBS], BF16)
    ge_d = nc.dram_tensor("ge_d", [R, G3], BF16)

    with tile.TileContext(nc) as tc, ExitStack() as ctx:
        # ----- cross-phase resident tiles -----
        consts = ctx.enter_context(tc.tile_pool(name="consts", bufs=1))
        va_s = consts.tile([128, KC, 128], BF16)
        nc.sync.dma_start(out=va_s, in_=P["vaTrep"][:, :].rearrange("(c p) n -> p c n", p=128))
        h0T_s = consts.tile([128, KC, BL], BF16)
        nc.sync.dma_start(out=h0T_s, in_=P["h0T"][:, :].rearrange("(c p) n -> p c n", p=128))
        bhn_s = consts.tile([BL, H], F32)
        nc.sync.dma_start(out=bhn_s, in_=P["bhn"][:, :])
        uab_s = consts.tile([128, KC], F32)
        nc.sync.dma_start(out=uab_s, in_=P["ua_bT"][:, :].rearrange("(c p) o -> p (c o)", p=128))
        wab_s = consts.tile([128, KC], F32)
        nc.sync.dma_start(out=wab_s, in_=P["wa_bT"][:, :].rearrange("(c p) o -> p (c o)", p=128))
        ident = consts.tile([128, 128], F32)
        make_identity(nc, ident)
        # h-state history: slot t holds h_{t+1}^T (bf16); chunk KC is the
        # constant ones-row used to fold out_b into the phase-2 matmul.
        hsT = consts.tile([128, KC + 1, R], BF16)
        nc.vector.memset(hsT[:, KC, :], 0.0)
        nc.vector.memset(hsT[0:1, KC, :], 1.0)

        # =================== phase 0: UkT and gi_emb ===================
        with tc.tile_pool(name="p0a", bufs=1) as p0a, \
             tc.tile_pool(name="p0aps", bufs=1, space="PSUM") as p0aps:
            ua_s = p0a.tile([128, KC, H], BF16)
            nc.sync.dma_start(out=ua_s, in_=P["uaT"][:, :].rearrange("(c p) n -> p c n", p=128))
            keys_s = p0a.tile([128, KC, NBS], BF16)
            nc.sync.dma_start(out=keys_s, in_=P["keysT"][:, :].rearrange("(c p) n -> p c n", p=128))
            for m in range(KC):          # output h-chunk of UkT
                uk_ps = p0aps.tile([128, 4, VT], F32, tag="ukps", bufs=2, name=f"ukps_{m}")
                for k in range(KC):      # n-inner: 4 matmuls per stationary
                    for n in range(4):
                        nc.tensor.matmul(uk_ps[:, n], ua_s[:, k, m * 128:(m + 1) * 128],
                                         keys_s[:, k, n * VT:(n + 1) * VT],
                                         start=(k == 0), stop=(k == KC - 1))
                uk_sb = p0a.tile([128, NBS], BF16, tag="uksb", bufs=2, name=f"uksb_{m}")
                nc.scalar.activation(uk_sb, uk_ps.rearrange("p a b -> p (a b)"),
                                     AF.Identity, bias=uab_s[:, m:m + 1])
                nc.sync.dma_start(out=ukT_d[m * 128:(m + 1) * 128, :], in_=uk_sb)

        with tc.tile_pool(name="p0b", bufs=1) as p0b, \
             tc.tile_pool(name="p0bps", bufs=1, space="PSUM") as p0bps:
            emb_s = p0b.tile([128, KC, R], BF16)
            nc.sync.dma_start(out=emb_s, in_=P["embT"][:, :].rearrange("(c p) n -> p c n", p=128))
            bfold_s = p0b.tile([128, G3], F32)
            nc.sync.dma_start(out=bfold_s, in_=P["bfold"][:, :])
            wihe_s = p0b.tile([128, KC, G3], BF16)
            nc.sync.dma_start(out=wihe_s, in_=P["wiheT"][:, :].rearrange("(c p) n -> p c n", p=128))
            for mi, (r0, rr) in enumerate([(0, 128), (128, 128), (256, 64)]):
                ge_ps = p0bps.tile([128, 6, VT], F32, tag="geps", bufs=1, name=f"geps_{mi}")
                for k in range(KC):
                    for n in range(6):
                        nc.tensor.matmul(ge_ps[:rr, n], emb_s[:, k, r0:r0 + rr],
                                         wihe_s[:, k, n * VT:(n + 1) * VT],
                                         start=(k == 0), stop=(k == KC - 1))
                ge_sb = p0b.tile([128, G3], BF16, tag="gesb", bufs=2, name=f"ge_{mi}")
                nc.vector.scalar_tensor_tensor(
                    out=ge_sb[:rr], in0=ge_ps[:rr].rearrange("p a b -> p (a b)"),
                    scalar=0.0, in1=bfold_s[:rr], op0=OP.add, op1=OP.add)
                nc.sync.dma_start(out=ge_d[r0:r0 + rr, :], in_=ge_sb[:rr])

        # =================== phase 1: 10 decode steps ===================
        with tc.tile_pool(name="p1", bufs=1) as p1, \
             tc.tile_pool(name="p1ps", bufs=1, space="PSUM") as p1ps:
            wihc_s = p1.tile([128, KC, G3], BF16)
            nc.sync.dma_start(out=wihc_s, in_=P["wihcT"][:, :].rearrange("(c p) n -> p c n", p=128))
            whh_s = p1.tile([128, KC, G3], BF16)
            nc.sync.dma_start(out=whh_s, in_=P["whhT"][:, :].rearrange("(c p) n -> p c n", p=128))
            h_f32 = None
            for t in range(T):
                def hk(k):
                    return h0T_s[:, k, :] if t == 0 else hsT[:, k, (t - 1) * BL:t * BL]

                # --- wq = h @ Wa^T, 2 col-groups (k 0-3 -> rows 0:32, 4-7 -> 32:64)
                wq_ps = p1ps.tile([64, 2, VT], F32, tag="small", bufs=2, name=f"wqps_{t}")
                for kh in range(4):   # stream waT 2 k-chunks at a time
                    wa_t = p1.tile([128, 2, H], BF16, tag="wat", bufs=2, name=f"wa_{t}_{kh}")
                    nc.sync.dma_start(out=wa_t, in_=P["waT"][kh * 256:(kh + 1) * 256, :]
                                      .rearrange("(c p) n -> p c n", p=128))
                    for kk in range(2):
                        k = kh * 2 + kk
                        g = k // 4
                        for n in range(2):
                            nc.tensor.matmul(wq_ps[g * BL:(g + 1) * BL, n], hk(k),
                                             wa_t[:, kk, n * VT:(n + 1) * VT],
                                             start=(k % 4 == 0), stop=(k % 4 == 3))
                wq_g1 = p1.tile([BL, H], F32, tag="gtmp", bufs=3, name=f"wqg1_{t}")
                nc.vector.tensor_copy(out=wq_g1, in_=wq_ps[BL:2 * BL].rearrange("p a b -> p (a b)"))
                wq_sb = p1.tile([BL, H], F32, tag="wqsb", bufs=1, name=f"wqsb_{t}")
                nc.vector.scalar_tensor_tensor(
                    out=wq_sb, in0=wq_ps[0:BL].rearrange("p a b -> p (a b)"), scalar=0.0,
                    in1=wq_g1, op0=OP.add, op1=OP.add)
                # transpose 8x [32,128] -> [128,32] through one 4-bank psum
                tr_ps = p1ps.tile([128, 4, VT], F32, tag="big", bufs=1, name=f"wqtr_{t}")
                for c in range(KC):
                    nc.tensor.transpose(
                        tr_ps[:, c // 2, (c % 2) * BL:((c % 2) + 1) * BL],
                        wq_sb[:, c * 128:(c + 1) * 128], ident[:BL, :BL])
                wqT = p1.tile([128, KC, BL], F32, tag="wqT", bufs=1, name=f"wqT_{t}")
                nc.scalar.activation(
                    wqT.rearrange("p (a h) c -> p a h c", h=2),
                    tr_ps[:, :, 0:2 * BL].rearrange("p a (h c) -> p a h c", h=2),
                    AF.Copy)  # Wa_b is folded into the tanh bias below

                # --- attention: e = tanh(Uk + wq + Wa_b), scores = Va.e (replicated) ---
                sc_ps = p1ps.tile([128, 4, VT], F32, tag="big", bufs=1, name=f"scps_{t}")
                for c in range(KC):
                    uk_t = p1.tile([128, NBS], BF16, tag="ukst", bufs=2, name=f"uk_{t}_{c}")
                    nc.sync.dma_start(out=uk_t, in_=ukT_d[c * 128:(c + 1) * 128, :])
                    pre = p1.tile([128, BL, S], BF16, tag="pre", bufs=2, name=f"pre_{t}_{c}")
                    nc.vector.scalar_tensor_tensor(
                        out=pre, in0=uk_t.rearrange("p (b s) -> p b s", b=BL),
                        scalar=0.0, in1=wqT[:, c, :].unsqueeze(-1).broadcast_to([128, BL, S]),
                        op0=OP.add, op1=OP.add)
                    e_sb = p1.tile([128, NBS], BF16, tag="esb", bufs=2, name=f"e_{t}_{c}")
                    nc.scalar.activation(e_sb, pre.rearrange("p b s -> p (b s)"), AF.Tanh,
                                         bias=wab_s[:, c:c + 1])
                    for n in range(4):
                        nc.tensor.matmul(sc_ps[:, n], va_s[:, c, :],
                                         e_sb[:, n * VT:(n + 1) * VT],
                                         start=(c == 0), stop=(c == KC - 1))
                # exp (no max subtraction needed: |scores| <= ~6)
                exp_sb = p1.tile([128, NBS], BF16, tag="expsb", bufs=1, name=f"exp_{t}")
                nc.scalar.activation(exp_sb, sc_ps.rearrange("p a b -> p (a b)"), AF.Exp)
                sums = p1.tile([128, BL], F32, tag="sums", bufs=2, name=f"sums_{t}")
                nc.vector.reduce_sum(sums, exp_sb.rearrange("p (b s) -> p b s", b=BL), axis=AX.X)
                recip = p1.tile([128, BL], F32, tag="recip", bufs=2, name=f"recip_{t}")
                nc.vector.reciprocal(recip, sums)

                # attentions output row (partition 0 only)
                attn_sb = p1.tile([1, NBS], F32, tag="rzsb2", bufs=1, name=f"attn_{t}")
                nc.vector.scalar_tensor_tensor(
                    out=attn_sb.rearrange("p (b s) -> p b s", b=BL),
                    in0=exp_sb[0:1, :].rearrange("p (b s) -> p b s", b=BL),
                    scalar=0.0, in1=recip[0:1, :].unsqueeze(-1).broadcast_to([1, BL, S]),
                    op0=OP.add, op1=OP.mult)
                nc.sync.dma_start(out=P["attnb"][t:t + 1, :], in_=attn_sb)

                # --- ctx^T[h, b] = sum_s keys*exp * recip ---
                ctx_red = p1.tile([128, KC, BL], F32, tag="ctxred", bufs=2, name=f"ctxred_{t}")
                for c in range(KC):
                    kt = p1.tile([128, NBS], BF16, tag="keyst", bufs=2, name=f"keys_{t}_{c}")
                    nc.sync.dma_start(out=kt, in_=P["keysT"][c * 128:(c + 1) * 128, :])
                    mt = p1.tile([128, NBS], BF16, tag="mult", bufs=2, name=f"mt_{t}_{c}")
                    nc.vector.tensor_tensor(out=mt, in0=kt, in1=exp_sb, op=OP.mult)
                    nc.vector.reduce_sum(ctx_red[:, c, :], mt.rearrange("p (b s) -> p b s", b=BL), axis=AX.X)
                ctxT = p1.tile([128, KC, BL], BF16, tag="ctxT", bufs=2, name=f"ctxT_{t}")
                nc.vector.tensor_tensor(out=ctxT, in0=ctx_red,
                                        in1=recip.unsqueeze(1).broadcast_to([128, KC, BL]),
                                        op=OP.mult)

                # --- GRU gates: 2 col-groups, n-inner for LDW amortization ---
                ge_t = p1.tile([BL, G3], BF16, tag="get", bufs=1, name=f"ge_{t}")
                nc.sync.dma_start(out=ge_t, in_=ge_d[t * BL:(t + 1) * BL, :])
                rz_sb = p1.tile([BL, 2 * H], F32, tag="rzsb2", bufs=1, name=f"rz_{t}")
                for half in range(2):
                    rz_ps = p1ps.tile([64, 2, VT], F32, tag="small", bufs=2, name=f"rzps_{t}_{half}")
                    for g in range(2):
                        for kk in range(4):
                            k = g * 4 + kk
                            j0 = half * 1024
                            for n in range(2):
                                nc.tensor.matmul(rz_ps[g * BL:(g + 1) * BL, n], ctxT[:, k, :],
                                                 wihc_s[:, k, j0 + n * VT:j0 + (n + 1) * VT],
                                                 start=(kk == 0), stop=False)
                            for n in range(2):
                                nc.tensor.matmul(rz_ps[g * BL:(g + 1) * BL, n], hk(k),
                                                 whh_s[:, k, j0 + n * VT:j0 + (n + 1) * VT],
                                                 start=False, stop=(kk == 3))
                    pre_rz = p1.tile([BL, H], F32, tag="gtmp", bufs=3, name=f"prerz_{t}_{half}")
                    nc.vector.scalar_tensor_tensor(
                        out=pre_rz, in0=rz_ps[BL:2 * BL].rearrange("p a b -> p (a b)"), scalar=0.0,
                        in1=ge_t[:, half * H:(half + 1) * H], op0=OP.add, op1=OP.add)
                    pre_rz2 = p1.tile([BL, H], F32, tag="gtmp", bufs=3, name=f"prerz2_{t}_{half}")
                    nc.vector.scalar_tensor_tensor(
                        out=pre_rz2, in0=rz_ps[0:BL].rearrange("p a b -> p (a b)"), scalar=0.0,
                        in1=pre_rz, op0=OP.add, op1=OP.add)
                    nc.scalar.activation(rz_sb[:, half * H:(half + 1) * H], pre_rz2, AF.Sigmoid)
                # inn / hn: j 2048:3072, 2 col-groups each
                in_ps = p1ps.tile([64, 2, VT], F32, tag="small", bufs=2, name=f"inps_{t}")
                hn_ps = p1ps.tile([64, 2, VT], F32, tag="small", bufs=2, name=f"hnps_{t}")
                for g in range(2):
                    for kk in range(4):
                        k = g * 4 + kk
                        for n in range(2):
                            j0 = 2 * H + n * VT
                            nc.tensor.matmul(in_ps[g * BL:(g + 1) * BL, n], ctxT[:, k, :],
                                             wihc_s[:, k, j0:j0 + VT],
                                             start=(kk == 0), stop=(kk == 3))
                        for n in range(2):
                            j0 = 2 * H + n * VT
                            nc.tensor.matmul(hn_ps[g * BL:(g + 1) * BL, n], hk(k),
                                             whh_s[:, k, j0:j0 + VT],
                                             start=(kk == 0), stop=(kk == 3))
                pre_in = p1.tile([BL, H], F32, tag="gtmp", bufs=3, name=f"prein_{t}")
                nc.vector.scalar_tensor_tensor(
                    out=pre_in, in0=in_ps[BL:2 * BL].rearrange("p a b -> p (a b)"), scalar=0.0,
                    in1=ge_t[:, 2 * H:], op0=OP.add, op1=OP.add)
                pre_in2 = p1.tile([BL, H], F32, tag="gtmp", bufs=3, name=f"prein2_{t}")
                nc.vector.scalar_tensor_tensor(
                    out=pre_in2, in0=in_ps[0:BL].rearrange("p a b -> p (a b)"), scalar=0.0,
                    in1=pre_in, op0=OP.add, op1=OP.add)
                pre_hn = p1.tile([BL, H], F32, tag="gtmp", bufs=3, name=f"prehn_{t}")
                nc.vector.scalar_tensor_tensor(
                    out=pre_hn, in0=hn_ps[BL:2 * BL].rearrange("p a b -> p (a b)"), scalar=0.0,
                    in1=bhn_s, op0=OP.add, op1=OP.add)
                pre_hn2 = p1.tile([BL, H], F32, tag="gtmp", bufs=3, name=f"prehn2_{t}")
                nc.vector.scalar_tensor_tensor(
                    out=pre_hn2, in0=hn_ps[0:BL].rearrange("p a b -> p (a b)"), scalar=0.0,
                    in1=pre_hn, op0=OP.add, op1=OP.add)
                # n = tanh(pre_in2 + r * pre_hn2)
                rhn = p1.tile([BL, H], F32, tag="gtmp", bufs=3, name=f"rhn_{t}")
                nc.vector.tensor_tensor(out=rhn, in0=rz_sb[:, :H], in1=pre_hn2, op=OP.mult)
                npre = p1.tile([BL, H], F32, tag="gtmp", bufs=3, name=f"npre_{t}")
                nc.vector.tensor_tensor(out=npre, in0=rhn, in1=pre_in2, op=OP.add)
                n_sb = p1.tile([BL, H], F32, tag="nsb", bufs=1, name=f"nsb_{t}")
                nc.scalar.activation(n_sb, npre, AF.Tanh)
                # h2 = n + z*(h - n)
                hprev_f = h_f32
                if t == 0:
                    hprev_f = p1.tile([BL, H], F32, tag="hf", bufs=2, name="hf_init")
                    nc.sync.dma_start(out=hprev_f, in_=P["h0f"][:, :])
                hmn = p1.tile([BL, H], F32, tag="gtmp", bufs=3, name=f"hmn_{t}")
                nc.vector.tensor_tensor(out=hmn, in0=hprev_f, in1=n_sb, op=OP.subtract)
                zh = p1.tile([BL, H], F32, tag="gtmp", bufs=3, name=f"zh_{t}")
                nc.vector.tensor_tensor(out=zh, in0=rz_sb[:, H:], in1=hmn, op=OP.mult)
                h_new = p1.tile([BL, H], F32, tag="hf", bufs=2, name=f"hf_{t}")
                nc.vector.tensor_tensor(out=h_new, in0=n_sb, in1=zh, op=OP.add)
                h_f32 = h_new
                # transpose h_new into hsT slot t (bf16), batched copy
                htr_ps = p1ps.tile([128, 4, VT], F32, tag="big", bufs=1, name=f"htr_{t}")
                for c in range(KC):
                    nc.tensor.transpose(
                        htr_ps[:, c // 2, (c % 2) * BL:((c % 2) + 1) * BL],
                        h_new[:, c * 128:(c + 1) * 128], ident[:BL, :BL])
                nc.scalar.activation(
                    hsT[:, 0:KC, t * BL:(t + 1) * BL].rearrange("p (a h) c -> p a h c", h=2),
                    htr_ps[:, :, 0:2 * BL].rearrange("p a (h c) -> p a h c", h=2),
                    AF.Copy)
                if t == T - 1:
                    nc.sync.dma_start(out=P["hlast"][:, :], in_=h_new)

        # =================== phase 2: logits + log_softmax ===================
        MT = [(0, 128), (128, 128), (256, 64)]
        NG = NV // 4   # 16 groups of 4 vocab tiles
        lg_d = nc.dram_tensor("lg_d", [R, VP], BF16)
        with tc.tile_pool(name="p2", bufs=1) as p2, \
             tc.tile_pool(name="p2ps", bufs=1, space="PSUM") as p2ps:
            s_acc = [p2.tile([mr, NG], F32, name=f"sacc{mi}")
                     for mi, (m0, mr) in enumerate(MT)]
            for ng in range(NG):
                ow_t = p2.tile([128, KC + 1, 4 * VT], BF16, tag="owt", bufs=3, name=f"ow_{ng}")
                nc.sync.dma_start(out=ow_t, in_=P["owT"][ng])
                for mi, (m0, mr) in enumerate(MT):
                    lg_ps = p2ps.tile([128, 4, VT], F32, tag="lgps", bufs=2, name=f"lgps_{ng}_{mi}")
                    for k in range(KC + 1):
                        for nn in range(4):
                            nc.tensor.matmul(lg_ps[:mr, nn], hsT[:, k, m0:m0 + mr],
                                             ow_t[:, k, nn * VT:(nn + 1) * VT],
                                             start=(k == 0), stop=(k == KC))
                    escr = p2.tile([128, 4 * VT], BF16, tag="escr", bufs=3, name=f"escr_{ng}_{mi}")
                    nc.scalar.activation(escr[:mr], lg_ps[:mr].rearrange("p a b -> p (a b)"),
                                         AF.Exp, accum_out=s_acc[mi][:, ng:ng + 1])
                    lg_sb = p2.tile([128, 4 * VT], BF16, tag="lgsb", bufs=4, name=f"lg_{ng}_{mi}")
                    nc.vector.tensor_copy(out=lg_sb[:mr], in_=lg_ps[:mr].rearrange("p a b -> p (a b)"))
                    nc.sync.dma_start(out=lg_d[m0:m0 + mr, ng * 4 * VT:(ng + 1) * 4 * VT],
                                      in_=lg_sb[:mr])
            # lse per m-tile, then dec = logits - lse (alternate ACT / DVE)
            for mi, (m0, mr) in enumerate(MT):
                ssum = p2.tile([mr, 1], F32, tag="ssum", bufs=2, name=f"ssum_{mi}")
                nc.vector.reduce_sum(ssum, s_acc[mi], axis=AX.X)
                nlse = p2.tile([mr, 1], F32, tag="nlse", bufs=2, name=f"nlse_{mi}")
                nc.scalar.activation(nlse, ssum, AF.Ln)
                nc.vector.tensor_scalar(out=nlse, in0=nlse, scalar1=-1.0, scalar2=None,
                                        op0=OP.mult)
                lse = p2.tile([mr, 1], F32, tag="lse", bufs=2, name=f"lse_{mi}")
                nc.vector.tensor_scalar(out=lse, in0=nlse, scalar1=-1.0, scalar2=None,
                                        op0=OP.mult)
                for ng in range(NG):
                    c0 = ng * 4 * VT
                    ncols = min(4 * VT, V - c0)
                    lg_in = p2.tile([128, 4 * VT], BF16, tag="lgin", bufs=6, name=f"lgi_{mi}_{ng}")
                    nc.sync.dma_start(out=lg_in[:mr, :ncols], in_=lg_d[m0:m0 + mr, c0:c0 + ncols])
                    dec_t = p2.tile([128, 4 * VT], F32, tag="dect", bufs=4, name=f"dec_{mi}_{ng}")
                    if ng % 2 == 0:
                        nc.scalar.activation(dec_t[:mr, :ncols], lg_in[:mr, :ncols],
                                             AF.Identity, bias=nlse)
                    else:
                        nc.vector.tensor_scalar(out=dec_t[:mr, :ncols], in0=lg_in[:mr, :ncols],
                                                scalar1=lse, scalar2=None, op0=OP.subtract)
                    nc.sync.dma_start(out=P["dec"][m0:m0 + mr, c0:c0 + ncols],
                                      in_=dec_t[:mr, :ncols])

    nc.finalize()
    return nc


def _prep(inputs):
    f32 = np.float32
    enc = np.asarray(inputs["encoder_outputs"], f32)          # [B, S, H]
    h0 = np.asarray(inputs["encoder_hidden"], f32)[0]         # [B, H]
    tgt = np.asarray(inputs["target_tensor"])                 # [B, T]
    emb = np.asarray(inputs["embedding"], f32)                # [V, H]
    Wa = np.asarray(inputs["Wa_w"], f32); Wab = np.asarray(inputs["Wa_b"], f32)
    Ua = np.asarray(inputs["Ua_w"], f32); Uab = np.asarray(inputs["Ua_b"], f32)
    Va = np.asarray(inputs["Va_w"], f32)
    wih = np.asarray(inputs["gru_w_ih"], f32); whh = np.asarray(inputs["gru_w_hh"], f32)
    bih = np.asarray(inputs["gru_b_ih"], f32); bhh = np.asarray(inputs["gru_b_hh"], f32)
    ow = np.asarray(inputs["out_w"], f32); ob = np.asarray(inputs["out_b"], f32)

    toks = np.concatenate([np.zeros((B, 1), tgt.dtype), tgt[:, :T - 1]], axis=1)  # [B, T]
    E = emb[toks.T]                                            # [T, B, H] gather

    uaT = np.ascontiguousarray(Ua.T).astype(BF)
    waT = np.ascontiguousarray(Wa.T).astype(BF)
    wihT = np.ascontiguousarray(wih.T)                         # [2H, 3H]
    wiheT = wihT[:H].astype(BF)
    wihcT = wihT[H:].astype(BF)
    whhT = np.ascontiguousarray(whh.T).astype(BF)
    vaTrep = np.repeat(Va[0][:, None], 128, axis=1).astype(BF)
    bfold = np.concatenate([(bih + bhh)[:2 * H], bih[2 * H:]])
    bfold_rep = np.tile(bfold[None, :], (128, 1)).astype(f32)
    bhn_rep = np.tile(bhh[2 * H:][None, :], (BL, 1)).astype(f32)

    OWP = np.zeros((KC + 1, 128, VP), BF)
    owT = np.ascontiguousarray(ow.T)                           # [H, V]
    OWP[:KC, :, :V] = owT.reshape(KC, 128, V).astype(BF)
    obp = np.full((VP,), -10000.0, f32)
    obp[:V] = ob
    OWP[KC, 0, :] = obp.astype(BF)
    owT_t = np.ascontiguousarray(
        OWP.reshape(KC + 1, 128, NV // 4, 4 * VT).transpose(2, 1, 0, 3))  # [NG, 128, KC+1, 2048]

    in_maps = []
    for c in range(NC):
        sl = slice(c * BL, (c + 1) * BL)
        keysT = np.ascontiguousarray(enc[sl].transpose(2, 0, 1).reshape(H, NBS)).astype(BF)
        embT = np.ascontiguousarray(E[:, sl].transpose(2, 0, 1).reshape(H, R)).astype(BF)
        h0c = np.ascontiguousarray(h0[sl])
        in_maps.append({
            "keysT": keysT, "uaT": uaT, "waT": waT, "wiheT": wiheT,
            "wihcT": wihcT, "whhT": whhT, "vaTrep": vaTrep, "embT": embT,
            "h0T": np.ascontiguousarray(h0c.T).astype(BF), "h0f": h0c,
            "ua_bT": Uab[:, None].astype(f32), "wa_bT": Wab[:, None].astype(f32),
            "bfold": bfold_rep, "bhn": bhn_rep, "owT": owT_t,
        })
    return in_maps


def _post(results):
    dec = np.stack([r["dec"].reshape(T, BL, V) for r in results])      # [NC, T, BL, V]
    dec = dec.transpose(0, 2, 1, 3).reshape(B, T, V)
    hlast = np.concatenate([r["hlast"] for r in results])[None]        # [1, B, H]
    attn = np.stack([r["attnb"].reshape(T, BL, S) for r in results])   # [NC, T, BL, S]
    attn = attn.transpose(0, 2, 1, 3).reshape(B, T, S)
    return dec, hlast, attn


def run_parts(inputs, trace=False, **kw):
    if "nc" not in _CACHE:
        _CACHE["nc"] = _build()
    nc = _CACHE["nc"]
    in_maps = _prep(inputs)
    res = run_bass_kernel_spmd(nc, in_maps, core_ids=list(range(NC)), trace=trace, **kw)
    return _post(res.results), res


def kernel(**inputs):
    (dec, hlast, attn), _ = run_parts(inputs)
    return dec, hlast, attn


# revision 17
# speedup vs baseline: 1.1558x; 1.0303x over previous
"""Trainium2 Bass kernel for nn_AttnDecoderRNN (Bahdanau attention + GRU decoder).

B=256, S=64, H=1024, V=32000, T=10 decode steps, 8 NeuronCores.
Sharding: pure data-parallel over batch (32 rows/core, zero collectives).

Per-core program:
  phase 0: UkT = (keys @ Ua^T + Ua_b)^T   (PE, bf16)  -> spilled to DRAM
           gi_emb = emb_all @ W_ih[:, :H]^T + b_fold  (PE)  -> spilled to DRAM
  phase 1: 10 sequential steps of additive attention + GRU (T-layout:
           h on partitions for attention; gates in [batch, j] layout)
  phase 2: logits = Hs @ out_w^T + out_b (bf16, out_b folded as a 9th
           K-chunk), online sum-exp for log_softmax (no max needed:
           logit range is +-3), then dec = logits - lse via ACT bias.

Host side does only layout work: shard, transpose, dtype-cast, embedding
GATHER (index lookup, no FLOPs), and final concat/permutes.
"""

import numpy as np
import ml_dtypes
from contextlib import ExitStack

import concourse.bass as bass
import concourse.bacc as bacc
import concourse.tile as tile
from concourse import mybir
from concourse.masks import make_identity
from concourse.alu_op_type import AluOpType
from concourse.bass_utils import run_bass_kernel_spmd

F32 = mybir.dt.float32
BF16 = mybir.dt.bfloat16
AF = mybir.ActivationFunctionType
AX = mybir.AxisListType
OP = AluOpType
BF = ml_dtypes.bfloat16

B, S, H, V, T = 256, 64, 1024, 32000, 10
NC = 8
BL = B // NC            # 32 batch rows per core
R = T * BL              # 320 decoded rows per core
KC = H // 128           # 8 k-chunks of the hidden dim
NBS = BL * S            # 2048 (b, s) pairs per core
VT = 512                # vocab tile
NV = 64                 # vocab tiles (padded V)
VP = NV * VT            # 32256 padded vocab
G3 = 3 * H              # 3072 gate width

_CACHE = {}


def _build():
    nc = bacc.Bacc(None, target_bir_lowering=False)
    P = {}

    def par(name, shape, dt, out=False):
        P[name] = nc.declare_dram_parameter(name, list(shape), dt, isOutput=out)
        return P[name]

    # ---- inputs (per core) ----
    par("keysT", [H, NBS], BF16)          # encoder outputs, (h, b*64+s)
    par("uaT", [H, H], BF16)              # Ua_w^T
    par("waT", [H, H], BF16)              # Wa_w^T
    par("wiheT", [H, G3], BF16)           # gru_w_ih[:, :H]^T
    par("wihcT", [H, G3], BF16)           # gru_w_ih[:, H:]^T
    par("whhT", [H, G3], BF16)            # gru_w_hh^T
    par("vaTrep", [H, 128], BF16)         # Va_w[0] replicated over 128 cols
    par("embT", [H, R], BF16)             # gathered embeddings, col = t*32+b
    par("h0T", [H, BL], BF16)
    par("h0f", [BL, H], F32)
    par("ua_bT", [H, 1], F32)
    par("wa_bT", [H, 1], F32)
    par("bfold", [128, G3], F32)          # (b_ih+b_hh)[:2H] ++ b_ih[2H:], row-rep
    par("bhn", [BL, H], F32)              # b_hh[2H:] replicated rows
    par("owT", [NV // 4, 128, KC + 1, 4 * VT], BF16)  # out_w^T tiled; chunk 8 row0=out_b
    # ---- outputs ----
    par("dec", [R, V], F32, out=True)
    par("hlast", [BL, H], F32, out=True)
    par("attnb", [T, NBS], F32, out=True)
    # ---- internal DRAM ----
    ukT_d = nc.dram_tensor("ukT_d", [H, NBS], BF16)
    ge_d = nc.dram_tensor("ge_d", [R, G3], BF16)

    with tile.TileContext(nc) as tc, ExitStack() as ctx:
        # ----- cross-phase resident tiles -----
        consts = ctx.enter_context(tc.tile_pool(name="consts", bufs=1))
        va_s = consts.tile([128, KC, 128], BF16)
        nc.sync.dma_start(out=va_s, in_=P["vaTrep"][:, :].rearrange("(c p) n -> p c n", p=128))
        h0T_s = consts.tile([128, KC, BL], BF16)
        nc.sync.dma_start(out=h0T_s, in_=P["h0T"][:, :].rearrange("(c p) n -> p c n", p=128))
        bhn_s = consts.tile([BL, H], F32)
        nc.sync.dma_start(out=bhn_s, in_=P["bhn"][:, :])
        uab_s = consts.tile([128, KC], F32)
        nc.sync.dma_start(out=uab_s, in_=P["ua_bT"][:, :].rearrange("(c p) o -> p (c o)", p=128))
        wab_s = consts.tile([128, KC], F32)
        nc.sync.dma_start(out=wab_s, in_=P["wa_bT"][:, :].rearrange("(c p) o -> p (c o)", p=128))
        ident = consts.tile([128, 128], F32)
        make_identity(nc, ident)
        # h-state history: slot t holds h_{t+1}^T (bf16); chunk KC is the
        # constant ones-row used to fold out_b into the phase-2 matmul.
        hsT = consts.tile([128, KC + 1, R], BF16)
        nc.vector.memset(hsT[:, KC, :], 0.0)
        nc.vector.memset(hsT[0:1, KC, :], 1.0)

        # =================== phase 0: UkT and gi_emb ===================
        with tc.tile_pool(name="p0a", bufs=1) as p0a, \
             tc.tile_pool(name="p0aps", bufs=1, space="PSUM") as p0aps:
            ua_s = p0a.tile([128, KC, H], BF16)
            nc.sync.dma_start(out=ua_s, in_=P["uaT"][:, :].rearrange("(c p) n -> p c n", p=128))
            keys_s = p0a.tile([128, KC, NBS], BF16)
            nc.sync.dma_start(out=keys_s, in_=P["keysT"][:, :].rearrange("(c p) n -> p c n", p=128))
            for m in range(KC):          # output h-chunk of UkT
                uk_ps = p0aps.tile([128, 4, VT], F32, tag="ukps", bufs=2, name=f"ukps_{m}")
                for k in range(KC):      # n-inner: 4 matmuls per stationary
                    for n in range(4):
                        nc.tensor.matmul(uk_ps[:, n], ua_s[:, k, m * 128:(m + 1) * 128],
                                         keys_s[:, k, n * VT:(n + 1) * VT],
                                         start=(k == 0), stop=(k == KC - 1))
                uk_sb = p0a.tile([128, NBS], BF16, tag="uksb", bufs=2, name=f"uksb_{m}")
                nc.scalar.activation(uk_sb, uk_ps.rearrange("p a b -> p (a b)"),
                                     AF.Identity, bias=uab_s[:, m:m + 1])
                nc.sync.dma_start(out=ukT_d[m * 128:(m + 1) * 128, :], in_=uk_sb)

        with tc.tile_pool(name="p0b", bufs=1) as p0b, \
             tc.tile_pool(name="p0bps", bufs=1, space="PSUM") as p0bps:
            emb_s = p0b.tile([128, KC, R], BF16)
            nc.sync.dma_start(out=emb_s, in_=P["embT"][:, :].rearrange("(c p) n -> p c n", p=128))
            bfold_s = p0b.tile([128, G3], F32)
            nc.sync.dma_start(out=bfold_s, in_=P["bfold"][:, :])
            wihe_s = p0b.tile([128, KC, G3], BF16)
            nc.sync.dma_start(out=wihe_s, in_=P["wiheT"][:, :].rearrange("(c p) n -> p c n", p=128))
            for mi, (r0, rr) in enumerate([(0, 128), (128, 128), (256, 64)]):
                ge_ps = p0bps.tile([128, 6, VT], F32, tag="geps", bufs=1, name=f"geps_{mi}")
                for k in range(KC):
                    for n in range(6):
                        nc.tensor.matmul(ge_ps[:rr, n], emb_s[:, k, r0:r0 + rr],
                                         wihe_s[:, k, n * VT:(n + 1) * VT],
                                         start=(k == 0), stop=(k == KC - 1))
                ge_sb = p0b.tile([128, G3], BF16, tag="gesb", bufs=2, name=f"ge_{mi}")
                nc.vector.scalar_tensor_tensor(
                    out=ge_sb[:rr], in0=ge_ps[:rr].rearrange("p a b -> p (a b)"),
                    scalar=0.0, in1=bfold_s[:rr], op0=OP.add, op1=OP.add)
                nc.sync.dma_start(out=ge_d[r0:r0 + rr, :], in_=ge_sb[:rr])

        # =================== phase 1: 10 decode steps ===================
        with tc.tile_pool(name="p1", bufs=1) as p1, \
             tc.tile_pool(name="p1ps", bufs=1, space="PSUM") as p1ps:
            wihc_s = p1.tile([128, KC, G3], BF16)
            nc.sync.dma_start(out=wihc_s, in_=P["wihcT"][:, :].rearrange("(c p) n -> p c n", p=128))
            whh_s = p1.tile([128, KC, G3], BF16)
            nc.sync.dma_start(out=whh_s, in_=P["whhT"][:, :].rearrange("(c p) n -> p c n", p=128))
            h_f32 = None
            for t in range(T):
                def hk(k):
                    return h0T_s[:, k, :] if t == 0 else hsT[:, k, (t - 1) * BL:t * BL]

                # ---- step-start PE burst: wq and hn (need only h) ----
                wq_ps = p1ps.tile([64, 2, VT], F32, tag="small", bufs=2, name=f"wqps_{t}")
                for kh in range(4):   # stream waT 2 k-chunks at a time
                    wa_t = p1.tile([128, 2, H], BF16, tag="wat", bufs=2, name=f"wa_{t}_{kh}")
                    nc.sync.dma_start(out=wa_t, in_=P["waT"][kh * 256:(kh + 1) * 256, :]
                                      .rearrange("(c p) n -> p c n", p=128))
                    for kk in range(2):
                        k = kh * 2 + kk
                        g = k // 4
                        for n in range(2):
                            nc.tensor.matmul(wq_ps[g * BL:(g + 1) * BL, n], hk(k),
                                             wa_t[:, kk, n * VT:(n + 1) * VT],
                                             start=(k % 4 == 0), stop=(k % 4 == 3))
                hn_ps = p1ps.tile([64, 2, VT], F32, tag="small", bufs=2, name=f"hnps_{t}")
                for g in range(2):
                    for kk in range(4):
                        k = g * 4 + kk
                        for n in range(2):
                            j0 = 2 * H + n * VT
                            nc.tensor.matmul(hn_ps[g * BL:(g + 1) * BL, n], hk(k),
                                             whh_s[:, k, j0:j0 + VT],
                                             start=(kk == 0), stop=(kk == 3))
                # hn pre-activation (independent of r): free the psum early
                pre_hn = p1.tile([BL, H], F32, tag="gtmp", bufs=3, name=f"prehn_{t}")
                nc.vector.scalar_tensor_tensor(
                    out=pre_hn, in0=hn_ps[BL:2 * BL].rearrange("p a b -> p (a b)"), scalar=0.0,
                    in1=bhn_s, op0=OP.add, op1=OP.add)
                pre_hn2 = p1.tile([BL, H], F32, tag="gtmp", bufs=3, name=f"prehn2_{t}")
                nc.vector.scalar_tensor_tensor(
                    out=pre_hn2, in0=hn_ps[0:BL].rearrange("p a b -> p (a b)"), scalar=0.0,
                    in1=pre_hn, op0=OP.add, op1=OP.add)
                # wq halves combine + transpose to wqT (bf16)
                wq_g1 = p1.tile([BL, H], F32, tag="gtmp", bufs=3, name=f"wqg1_{t}")
                nc.vector.tensor_copy(out=wq_g1, in_=wq_ps[BL:2 * BL].rearrange("p a b -> p (a b)"))
                wq_sb = p1.tile([BL, H], F32, tag="wqsb", bufs=1, name=f"wqsb_{t}")
                nc.vector.scalar_tensor_tensor(
                    out=wq_sb, in0=wq_ps[0:BL].rearrange("p a b -> p (a b)"), scalar=0.0,
                    in1=wq_g1, op0=OP.add, op1=OP.add)
                tr_ps = p1ps.tile([128, 4, VT], F32, tag="big", bufs=1, name=f"wqtr_{t}")
                for c in range(KC):
                    nc.tensor.transpose(
                        tr_ps[:, c // 2, (c % 2) * BL:((c % 2) + 1) * BL],
                        wq_sb[:, c * 128:(c + 1) * 128], ident[:BL, :BL])
                wqT = p1.tile([128, KC, BL], BF16, tag="wqT", bufs=1, name=f"wqT_{t}")
                nc.scalar.activation(
                    wqT.rearrange("p (a h) c -> p a h c", h=2),
                    tr_ps[:, :, 0:2 * BL].rearrange("p a (h c) -> p a h c", h=2),
                    AF.Copy)  # Wa_b is folded into the tanh bias below

                # --- attention (2-chunk pairs): e = tanh(Uk + wq + Wa_b) ---
                sc_ps = p1ps.tile([128, 4, VT], F32, tag="big", bufs=1, name=f"scps_{t}")
                for cp in range(4):
                    uk_t = p1.tile([128, 2, NBS], BF16, tag="ukst", bufs=2, name=f"uk_{t}_{cp}")
                    nc.sync.dma_start(out=uk_t, in_=ukT_d[cp * 256:(cp + 1) * 256, :]
                                      .rearrange("(c p) n -> p c n", p=128))
                    nc.vector.scalar_tensor_tensor(
                        out=uk_t.rearrange("p c (b s) -> p c b s", b=BL),
                        in0=uk_t.rearrange("p c (b s) -> p c b s", b=BL),
                        scalar=0.0,
                        in1=wqT[:, 2 * cp:2 * cp + 2, :].unsqueeze(-1).broadcast_to([128, 2, BL, S]),
                        op0=OP.add, op1=OP.add)
                    for cc in range(2):
                        c = 2 * cp + cc
                        e_sb = p1.tile([128, NBS], BF16, tag="esb", bufs=2, name=f"e_{t}_{cp}_{cc}")
                        nc.scalar.activation(e_sb, uk_t[:, cc],
                                             AF.Tanh, bias=wab_s[:, c:c + 1])
                        for n in range(4):
                            nc.tensor.matmul(sc_ps[:, n], va_s[:, c, :],
                                             e_sb[:, n * VT:(n + 1) * VT],
                                             start=(c == 0), stop=(c == KC - 1))
                # exp (no max subtraction needed: |scores| <= ~6)
                exp_sb = p1.tile([128, NBS], BF16, tag="expsb", bufs=1, name=f"exp_{t}")
                nc.scalar.activation(exp_sb, sc_ps.rearrange("p a b -> p (a b)"), AF.Exp)
                sums = p1.tile([128, BL], BF16, tag="sums", bufs=2, name=f"sums_{t}")
                with nc.allow_low_precision(reason="rowsum of 64 bf16 exps; 2e-2 gate"):
                    nc.vector.reduce_sum(sums, exp_sb.rearrange("p (b s) -> p b s", b=BL), axis=AX.X)
                recip = p1.tile([128, BL], F32, tag="recip", bufs=2, name=f"recip_{t}")
                nc.vector.reciprocal(recip, sums)

                # attentions output row (partition 0 only) on GpSimd
                attn_sb = p1.tile([1, NBS], F32, tag="rzsb2", bufs=1, name=f"attn_{t}")
                nc.vector.scalar_tensor_tensor(
                    out=attn_sb.rearrange("p (b s) -> p b s", b=BL),
                    in0=exp_sb[0:1, :].rearrange("p (b s) -> p b s", b=BL),
                    scalar=0.0, in1=recip[0:1, :].unsqueeze(-1).broadcast_to([1, BL, S]),
                    op0=OP.add, op1=OP.mult)
                nc.sync.dma_start(out=P["attnb"][t:t + 1, :], in_=attn_sb)

                # --- ctx^T[h, b] = sum_s keys*exp * recip (2-chunk pairs) ---
                ctx_red = p1.tile([128, KC, BL], BF16, tag="ctxred", bufs=2, name=f"ctxred_{t}")
                for cp in range(4):
                    kt = p1.tile([128, 2, NBS], BF16, tag="keyst", bufs=2, name=f"keys_{t}_{cp}")
                    nc.sync.dma_start(out=kt, in_=P["keysT"][cp * 256:(cp + 1) * 256, :]
                                      .rearrange("(c p) n -> p c n", p=128))
                    nc.vector.tensor_tensor(
                        out=kt, in0=kt,
                        in1=exp_sb.unsqueeze(1).broadcast_to([128, 2, NBS]), op=OP.mult)
                    with nc.allow_low_precision(reason="attn ctx rowsum; 2e-2 gate"):
                        nc.vector.reduce_sum(ctx_red[:, 2 * cp:2 * cp + 2, :],
                                             kt.rearrange("p c (b s) -> p c b s", b=BL), axis=AX.X)
                ctxT = p1.tile([128, KC, BL], BF16, tag="ctxT", bufs=2, name=f"ctxT_{t}")
                nc.vector.tensor_tensor(out=ctxT, in0=ctx_red,
                                        in1=recip.unsqueeze(1).broadcast_to([128, KC, BL]),
                                        op=OP.mult)

                # --- GRU gates rz + inn ---
                ge_t = p1.tile([BL, G3], BF16, tag="get", bufs=1, name=f"ge_{t}")
                nc.sync.dma_start(out=ge_t, in_=ge_d[t * BL:(t + 1) * BL, :])
                rz_sb = p1.tile([BL, 2 * H], F32, tag="rzsb2", bufs=1, name=f"rz_{t}")
                for half in range(2):
                    rz_ps = p1ps.tile([64, 2, VT], F32, tag="small", bufs=2, name=f"rzps_{t}_{half}")
                    for g in range(2):
                        for kk in range(4):
                            k = g * 4 + kk
                            j0 = half * 1024
                            for n in range(2):
                                nc.tensor.matmul(rz_ps[g * BL:(g + 1) * BL, n], hk(k),
                                                 whh_s[:, k, j0 + n * VT:j0 + (n + 1) * VT],
                                                 start=(kk == 0), stop=False)
                            for n in range(2):
                                nc.tensor.matmul(rz_ps[g * BL:(g + 1) * BL, n], ctxT[:, k, :],
                                                 wihc_s[:, k, j0 + n * VT:j0 + (n + 1) * VT],
                                                 start=False, stop=(kk == 3))
                    pre_rz = p1.tile([BL, H], F32, tag="gtmp", bufs=3, name=f"prerz_{t}_{half}")
                    nc.vector.scalar_tensor_tensor(
                        out=pre_rz, in0=rz_ps[BL:2 * BL].rearrange("p a b -> p (a b)"), scalar=0.0,
                        in1=ge_t[:, half * H:(half + 1) * H], op0=OP.add, op1=OP.add)
                    pre_rz2 = p1.tile([BL, H], F32, tag="gtmp", bufs=3, name=f"prerz2_{t}_{half}")
                    nc.vector.scalar_tensor_tensor(
                        out=pre_rz2, in0=rz_ps[0:BL].rearrange("p a b -> p (a b)"), scalar=0.0,
                        in1=pre_rz, op0=OP.add, op1=OP.add)
                    nc.scalar.activation(rz_sb[:, half * H:(half + 1) * H], pre_rz2, AF.Sigmoid)
                in_ps = p1ps.tile([64, 2, VT], F32, tag="small", bufs=2, name=f"inps_{t}")
                for g in range(2):
                    for kk in range(4):
                        k = g * 4 + kk
                        for n in range(2):
                            j0 = 2 * H + n * VT
                            nc.tensor.matmul(in_ps[g * BL:(g + 1) * BL, n], ctxT[:, k, :],
                                             wihc_s[:, k, j0:j0 + VT],
                                             start=(kk == 0), stop=(kk == 3))
                pre_in = p1.tile([BL, H], F32, tag="gtmp", bufs=3, name=f"prein_{t}")
                nc.vector.scalar_tensor_tensor(
                    out=pre_in, in0=in_ps[BL:2 * BL].rearrange("p a b -> p (a b)"), scalar=0.0,
                    in1=ge_t[:, 2 * H:], op0=OP.add, op1=OP.add)
                pre_in2 = p1.tile([BL, H], F32, tag="gtmp", bufs=3, name=f"prein2_{t}")
                nc.vector.scalar_tensor_tensor(
                    out=pre_in2, in0=in_ps[0:BL].rearrange("p a b -> p (a b)"), scalar=0.0,
                    in1=pre_in, op0=OP.add, op1=OP.add)
                # n = tanh(pre_in2 + r * pre_hn2)
                rhn = p1.tile([BL, H], F32, tag="gtmp", bufs=3, name=f"rhn_{t}")
                nc.vector.tensor_tensor(out=rhn, in0=rz_sb[:, :H], in1=pre_hn2, op=OP.mult)
                npre = p1.tile([BL, H], F32, tag="gtmp", bufs=3, name=f"npre_{t}")
                nc.vector.tensor_tensor(out=npre, in0=rhn, in1=pre_in2, op=OP.add)
                n_sb = p1.tile([BL, H], F32, tag="nsb", bufs=1, name=f"nsb_{t}")
                nc.scalar.activation(n_sb, npre, AF.Tanh)
                # h2 = n + z*(h - n)
                hprev_f = h_f32
                if t == 0:
                    hprev_f = p1.tile([BL, H], F32, tag="hf", bufs=2, name="hf_init")
                    nc.sync.dma_start(out=hprev_f, in_=P["h0f"][:, :])
                hmn = p1.tile([BL, H], F32, tag="gtmp", bufs=3, name=f"hmn_{t}")
                nc.vector.tensor_tensor(out=hmn, in0=hprev_f, in1=n_sb, op=OP.subtract)
                zh = p1.tile([BL, H], F32, tag="gtmp", bufs=3, name=f"zh_{t}")
                nc.vector.tensor_tensor(out=zh, in0=rz_sb[:, H:], in1=hmn, op=OP.mult)
                h_new = p1.tile([BL, H], F32, tag="hf", bufs=2, name=f"hf_{t}")
                nc.vector.tensor_tensor(out=h_new, in0=n_sb, in1=zh, op=OP.add)
                h_f32 = h_new
                # transpose h_new into hsT slot t (bf16), batched copy
                htr_ps = p1ps.tile([128, 4, VT], F32, tag="big", bufs=1, name=f"htr_{t}")
                for c in range(KC):
                    nc.tensor.transpose(
                        htr_ps[:, c // 2, (c % 2) * BL:((c % 2) + 1) * BL],
                        h_new[:, c * 128:(c + 1) * 128], ident[:BL, :BL])
                nc.scalar.activation(
                    hsT[:, 0:KC, t * BL:(t + 1) * BL].rearrange("p (a h) c -> p a h c", h=2),
                    htr_ps[:, :, 0:2 * BL].rearrange("p a (h c) -> p a h c", h=2),
                    AF.Copy)
                if t == T - 1:
                    nc.sync.dma_start(out=P["hlast"][:, :], in_=h_new)

        # =================== phase 2: logits + log_softmax ===================
        MT = [(0, 128), (128, 128), (256, 64)]
        NG = NV // 4   # 16 groups of 4 vocab tiles
        lg_d = nc.dram_tensor("lg_d", [R, VP], BF16)
        with tc.tile_pool(name="p2", bufs=1) as p2, \
             tc.tile_pool(name="p2ps", bufs=1, space="PSUM") as p2ps:
            s_acc = [p2.tile([mr, NG], F32, name=f"sacc{mi}")
                     for mi, (m0, mr) in enumerate(MT)]
            for ng in range(NG):
                ow_t = p2.tile([128, KC + 1, 4 * VT], BF16, tag="owt", bufs=2, name=f"ow_{ng}")
                nc.sync.dma_start(out=ow_t, in_=P["owT"][ng])
                for mi, (m0, mr) in enumerate(MT):
                    lg_ps = p2ps.tile([128, 4, VT], F32, tag="lgps", bufs=2, name=f"lgps_{ng}_{mi}")
                    for k in range(KC + 1):
                        for nn in range(4):
                            nc.tensor.matmul(lg_ps[:mr, nn], hsT[:, k, m0:m0 + mr],
                                             ow_t[:, k, nn * VT:(nn + 1) * VT],
                                             start=(k == 0), stop=(k == KC))
                    escr = p2.tile([128, 4 * VT], BF16, tag="escr", bufs=3, name=f"escr_{ng}_{mi}")
                    nc.scalar.activation(escr[:mr], lg_ps[:mr].rearrange("p a b -> p (a b)"),
                                         AF.Exp, accum_out=s_acc[mi][:, ng:ng + 1])
                    lg_sb = p2.tile([128, 4 * VT], BF16, tag="lgsb", bufs=4, name=f"lg_{ng}_{mi}")
                    nc.vector.tensor_copy(out=lg_sb[:mr], in_=lg_ps[:mr].rearrange("p a b -> p (a b)"))
                    nc.sync.dma_start(out=lg_d[m0:m0 + mr, ng * 4 * VT:(ng + 1) * 4 * VT],
                                      in_=lg_sb[:mr])
            # lse per m-tile, then dec = logits - lse (alternate ACT / DVE)
            for mi, (m0, mr) in enumerate(MT):
                ssum = p2.tile([mr, 1], F32, tag="ssum", bufs=2, name=f"ssum_{mi}")
                nc.vector.reduce_sum(ssum, s_acc[mi], axis=AX.X)
                nlse = p2.tile([mr, 1], F32, tag="nlse", bufs=2, name=f"nlse_{mi}")
                nc.scalar.activation(nlse, ssum, AF.Ln)
                nc.vector.tensor_scalar(out=nlse, in0=nlse, scalar1=-1.0, scalar2=None,
                                        op0=OP.mult)
                lse = p2.tile([mr, 1], F32, tag="lse", bufs=2, name=f"lse_{mi}")
                nc.vector.tensor_scalar(out=lse, in0=nlse, scalar1=-1.0, scalar2=None,
                                        op0=OP.mult)
                for ng in range(NG):
                    c0 = ng * 4 * VT
                    ncols = min(4 * VT, V - c0)
                    lg_in = p2.tile([128, 4 * VT], BF16, tag="lgin", bufs=12, name=f"lgi_{mi}_{ng}")
                    nc.sync.dma_start(out=lg_in[:mr, :ncols], in_=lg_d[m0:m0 + mr, c0:c0 + ncols])
                    dec_t = p2.tile([128, 4 * VT], F32, tag="dect", bufs=4, name=f"dec_{mi}_{ng}")
                    if ng % 2 == 0:
                        nc.scalar.activation(dec_t[:mr, :ncols], lg_in[:mr, :ncols],
                                             AF.Identity, bias=nlse)
                    else:
                        nc.vector.tensor_scalar(out=dec_t[:mr, :ncols], in0=lg_in[:mr, :ncols],
                                                scalar1=lse, scalar2=None, op0=OP.subtract)
                    nc.sync.dma_start(out=P["dec"][m0:m0 + mr, c0:c0 + ncols],
                                      in_=dec_t[:mr, :ncols])

    nc.finalize()
    return nc


def _prep(inputs):
    f32 = np.float32
    enc = np.asarray(inputs["encoder_outputs"], f32)          # [B, S, H]
    h0 = np.asarray(inputs["encoder_hidden"], f32)[0]         # [B, H]
    tgt = np.asarray(inputs["target_tensor"])                 # [B, T]
    emb = np.asarray(inputs["embedding"], f32)                # [V, H]
    Wa = np.asarray(inputs["Wa_w"], f32); Wab = np.asarray(inputs["Wa_b"], f32)
    Ua = np.asarray(inputs["Ua_w"], f32); Uab = np.asarray(inputs["Ua_b"], f32)
    Va = np.asarray(inputs["Va_w"], f32)
    wih = np.asarray(inputs["gru_w_ih"], f32); whh = np.asarray(inputs["gru_w_hh"], f32)
    bih = np.asarray(inputs["gru_b_ih"], f32); bhh = np.asarray(inputs["gru_b_hh"], f32)
    ow = np.asarray(inputs["out_w"], f32); ob = np.asarray(inputs["out_b"], f32)

    toks = np.concatenate([np.zeros((B, 1), tgt.dtype), tgt[:, :T - 1]], axis=1)  # [B, T]
    E = emb[toks.T]                                            # [T, B, H] gather

    uaT = np.ascontiguousarray(Ua.T).astype(BF)
    waT = np.ascontiguousarray(Wa.T).astype(BF)
    wihT = np.ascontiguousarray(wih.T)                         # [2H, 3H]
    wiheT = wihT[:H].astype(BF)
    wihcT = wihT[H:].astype(BF)
    whhT = np.ascontiguousarray(whh.T).astype(BF)
    vaTrep = np.repeat(Va[0][:, None], 128, axis=1).astype(BF)
    bfold = np.concatenate([(bih + bhh)[:2 * H], bih[2 * H:]])
    bfold_rep = np.tile(bfold[None, :], (128, 1)).astype(f32)
    bhn_rep = np.tile(bhh[2 * H:][None, :], (BL, 1)).astype(f32)

    OWP = np.zeros((KC + 1, 128, VP), BF)
    owT = np.ascontiguousarray(ow.T)                           # [H, V]
    OWP[:KC, :, :V] = owT.reshape(KC, 128, V).astype(BF)
    obp = np.full((VP,), -10000.0, f32)
    obp[:V] = ob
    OWP[KC, 0, :] = obp.astype(BF)
    owT_t = np.ascontiguousarray(
        OWP.reshape(KC + 1, 128, NV // 4, 4 * VT).transpose(2, 1, 0, 3))  # [NG, 128, KC+1, 2048]

    in_maps = []
    for c in range(NC):
        sl = slice(c * BL, (c + 1) * BL)
        keysT = np.ascontiguousarray(enc[sl].transpose(2, 0, 1).reshape(H, NBS)).astype(BF)
        embT = np.ascontiguousarray(E[:, sl].transpose(2, 0, 1).reshape(H, R)).astype(BF)
        h0c = np.ascontiguousarray(h0[sl])
        in_maps.append({
            "keysT": keysT, "uaT": uaT, "waT": waT, "wiheT": wiheT,
            "wihcT": wihcT, "whhT": whhT, "vaTrep": vaTrep, "embT": embT,
            "h0T": np.ascontiguousarray(h0c.T).astype(BF), "h0f": h0c,
            "ua_bT": Uab[:, None].astype(f32), "wa_bT": Wab[:, None].astype(f32),
            "bfold": bfold_rep, "bhn": bhn_rep, "owT": owT_t,
        })
    return in_maps


def _post(results):
    dec = np.stack([r["dec"].reshape(T, BL, V) for r in results])      # [NC, T, BL, V]
    dec = dec.transpose(0, 2, 1, 3).reshape(B, T, V)
    hlast = np.concatenate([r["hlast"] for r in results])[None]        # [1, B, H]
    attn = np.stack([r["attnb"].reshape(T, BL, S) for r in results])   # [NC, T, BL, S]
    attn = attn.transpose(0, 2, 1, 3).reshape(B, T, S)
    return dec, hlast, attn


def run_parts(inputs, trace=False, **kw):
    if "nc" not in _CACHE:
        _CACHE["nc"] = _build()
    nc = _CACHE["nc"]
    in_maps = _prep(inputs)
    res = run_bass_kernel_spmd(nc, in_maps, core_ids=list(range(NC)), trace=trace, **kw)
    return _post(res.results), res


def kernel(**inputs):
    (dec, hlast, attn), _ = run_parts(inputs)
    return dec, hlast, attn
